# revision 24
# baseline (speedup 1.0000x reference)
"""Trainium2 Bass kernel for nn_Aggregation_Separation_Loss.

Math: pairwise SmoothL1 (beta=1, mean over D) for all (i,j):
    huber(z) = 0.5*z^2 - 0.5*relu(|z|-1)^2
    sl1[i,j]*D = 0.5*s_i + 0.5*s_j - G_ij - 0.5*V_ij
with s_i = ||x_i||^2, G = X X^T (TensorE matmul), and
V_ij = sum_d relu(|x_id-x_jd|-1)^2 the only O(N^2 D) elementwise part.

Key symmetry: relu(|z|-1)^2 = relu(z-1)^2 + relu(-z-1)^2, and the second
term of pair (i,j) is the first term of (j,i).  So with the one-sided
P_ij = sum_d relu(x_i - x_j - 1)^2 over ALL ordered pairs:
    sum_{S symmetric} 0.5*V = sum_S P   (diag: P_ii = 0).
Per (row i, d-tile of 128) unit:
    DVE :  u = (Xt - x_i) max 1.0      (one fused tensor_scalar, 2x mode)
    ACT :  v = Square(u - 1) = relu(z-1)^2    (bias = -1)
    PE  :  selector-matmul contracts v over d into PSUM row i -> P[i, :]
G goes to a second PSUM tile; on-device masked reductions give per core
    SA = sum_{same-label ordered} (G + P),  SB = sum_{all ordered} (G + P)
and the host finishes with closed forms in f64:
    inner_sum = (sum_c N_c*S_c - SA_tot) / D
    total_sum = (N*sum(s)  - SB_tot) / D.

Sharding: plain row-parallel, core c owns rows [96c, 96c+96); every unit
is full width so the SPMD program is uniform across cores.
"""

import numpy as np

import concourse.bass as bass
import concourse.mybir as mybir
import concourse.tile as tile
from concourse.bacc import Bacc

N = 768
D = 256
NCORES = 8
ROWS = 96
DT = 2  # d-tiles of 128 partitions
F32 = mybir.dt.float32
BF16 = mybir.dt.bfloat16
# big input column layout
XTC0 = 2 * N                      # 1536
WSEL0 = XTC0 + 2 * ROWS           # 1728
ONES0 = WSEL0 + 1024              # 2752
AM0 = ONES0 + 1                   # 2753
XTCP0 = AM0 + N                   # 3521: xtc+1 (for relu(z-1) units)
BW = XTCP0 + 2 * ROWS + 1         # 3714 (even)

_NC_CACHE = {}


def _chunks():
    """Split [0, N) at the PSUM bank boundary (512 f32)."""
    return [(0, 512), (512, N)]


def build_nc(heavy_reps=1):
    # heavy_reps > 1 repeats the heavy loop for wall-clock timing via
    # repetition amplification (output is then wrong; timing only).
    #
    # Bacc (not plain Bass): its finalize() runs move_matmul_waits_to_
    # ldweights + generate_event_semaphores, which legalize multi-wait
    # instructions down to the 1-sync-wait-per-instruction HW limit.
    #
    # All inputs ride in ONE dram tensor / one DMA so consumers need few
    # cross-engine waits.  Column layout:
    #   0:768      xt d-tile 0          768:1536   xt d-tile 1
    #   1536:1632  xtc d-tile 0         1632:1728  xtc d-tile 1
    #   1728:2752  wsel (32 selector variants)
    #   2752:2753  ones column (rows 0:96)
    #   2753:3521  am mask (rows 0:96)
    nc = Bacc()
    big_d = nc.dram_tensor("big", [128, BW], F32, kind="ExternalInput")
    wselb_d = nc.dram_tensor("wselb", [128, 1024], BF16, kind="ExternalInput")
    out_d = nc.dram_tensor("out", [1, 2], F32, kind="ExternalOutput")

    with tile.TileContext(nc) as tc:
        with (
            tc.tile_pool(name="pers", bufs=1) as pers,
            tc.tile_pool(name="tt", bufs=6) as tpool,
            tc.tile_pool(name="vv", bufs=6) as vpool,
            tc.tile_pool(name="fin", bufs=1) as fin,
            tc.tile_pool(name="psum", bufs=1, space=bass.MemorySpace.PSUM) as psum,
        ):
            big = pers.tile([128, BW], F32, tag="big")
            wselb = pers.tile([128, 1024], BF16, tag="wselb")
            nc.gpsimd.dma_start(big[:], big_d[:])
            nc.gpsimd.dma_start(wselb[:], wselb_d[:])

            xt = [big[:, 0:N], big[:, N : 2 * N]]
            xtc = [
                big[:, XTC0 : XTC0 + ROWS],
                big[:, XTC0 + ROWS : XTC0 + 2 * ROWS],
            ]
            # wsel[:, 32r:32r+32] is a [128,32] selector: column r = 1.0,
            # rest 0.  As matmul lhsT it adds sum_d(rhs) into psum row r of
            # a 32-row block and 0 into the other 31 rows (PE psum outputs
            # must start at partition 0/32/64, so m=1 writes at arbitrary
            # partitions are not allowed).
            ones96 = big[0:ROWS, ONES0 : ONES0 + 1]
            am = big[0:ROWS, AM0 : AM0 + N]
            xtcp = [
                big[:, XTCP0 : XTCP0 + ROWS],
                big[:, XTCP0 + ROWS : XTCP0 + 2 * ROWS],
            ]

            pg = psum.tile([ROWS, N], F32, tag="pg")
            pv = psum.tile([ROWS, N], F32, tag="pv")
            sab = psum.tile([1, 2], F32, tag="sab")

            # G = X_rows @ X^T accumulated over the two d-tiles.
            for c0, c1 in _chunks():
                for t in range(DT):
                    nc.tensor.matmul(
                        pg[:, c0:c1],
                        xtc[t][:],
                        xt[t][:, c0:c1],
                        start=(t == 0),
                        stop=(t == DT - 1),
                    )

            # Heavy loop: P rows into pv.  Per unit (row, d-tile):
            #   u = relu(x_j - x_i - 1) = (xt - (x_i+1)) max 0   [DVE or POOL]
            #   v = u*u  (bf16)                                  [ACT or POOL]
            #   selector-matmul v over d into psum row           [PE]
            # Engines are mixed per unit to balance DVE/ACT/POOL loads
            # (approx. costs per [128,768] op: DVE ts 460ns, POOL 640ns,
            # ACT square 825ns, PE ~330ns).
            first_touch = set()
            for rep in range(heavy_reps):
                for row in range(ROWS):
                    b, r = divmod(row, 32)
                    for t in range(DT):
                        idx = 2 * row + t
                        tt = tpool.tile(
                            [128, N], F32, tag="tt", name=f"tt_{rep}_{row}_{t}"
                        )
                        tt_eng = (
                            nc.gpsimd if idx % 8 == 7 else nc.vector
                        )
                        tt_eng.tensor_scalar(
                            tt[:],
                            xt[t][:],
                            xtcp[t][:, row : row + 1],
                            0.0,
                            op0=mybir.AluOpType.subtract,
                            op1=mybir.AluOpType.max,
                        )
                        vv = vpool.tile(
                            [128, N], BF16, tag="vv", name=f"vv_{rep}_{row}_{t}"
                        )
                        if idx % 2 == 0:
                            nc.scalar.activation(
                                vv[:],
                                tt[:],
                                mybir.ActivationFunctionType.Square,
                                bias=0.0,
                                scale=1.0,
                            )
                        else:
                            nc.gpsimd.tensor_tensor(
                                vv[:], tt[:], tt[:], op=mybir.AluOpType.mult
                            )
                        for c0, c1 in _chunks():
                            # start=True resets the full 32-row psum block,
                            # so only the first touch of each (block, chunk)
                            # region may use it.
                            key = (b, c0)
                            nc.tensor.matmul(
                                pv[32 * b : 32 * b + 32, c0:c1],
                                wselb[:, 32 * r : 32 * r + 32],
                                vv[:, c0:c1],
                                start=key not in first_touch,
                                stop=(rep == heavy_reps - 1 and t == DT - 1),
                                skip_group_check=True,
                            )
                            first_touch.add(key)

            # R = G + P ; SA = sum(R*am) ; SB = sum(R)
            # (TT can read at most one input from PSUM -> stage pv in SBUF.)
            r0 = fin.tile([ROWS, N], F32, tag="r0")
            nc.vector.tensor_scalar(r0[:], pv[:], 1.0, None, op0=mybir.AluOpType.mult)
            r1 = fin.tile([ROWS, N], F32, tag="r1")
            nc.vector.tensor_tensor(r1[:], r0[:], pg[:], op=mybir.AluOpType.add)

            red = fin.tile([ROWS, 2], F32, tag="red")
            prod = fin.tile([ROWS, N], F32, tag="prod")
            nc.vector.tensor_tensor(prod[:], r1[:], am[:], op=mybir.AluOpType.mult)
            nc.vector.tensor_reduce(
                red[:, 0:1], prod[:], axis=mybir.AxisListType.X,
                op=mybir.AluOpType.add,
            )
            nc.vector.tensor_reduce(
                red[:, 1:2], r1[:], axis=mybir.AxisListType.X,
                op=mybir.AluOpType.add,
            )
            nc.tensor.matmul(sab[:], ones96[:], red[:], start=True, stop=True)

            outsb = fin.tile([1, 2], F32, tag="outsb")
            nc.scalar.copy(outsb[:], sab[:])
            nc.gpsimd.dma_start(out_d[:], outsb[:])

    nc.finalize()
    return nc


def core_rows(c):
    return np.arange(ROWS * c, ROWS * (c + 1))


def prepare_in_maps(X, lab):
    """X: [N, D] f32, lab: [N] int -> list of per-core input dicts."""
    import ml_dtypes

    XT = np.ascontiguousarray(X.T)  # [D, N]
    wselb = np.zeros((128, 1024), dtype=ml_dtypes.bfloat16)
    for r in range(32):
        wselb[:, 32 * r + r] = 1.0
    in_maps = []
    for c in range(NCORES):
        rows = core_rows(c)
        big = np.zeros((128, BW), np.float32)
        big[:, 0:N] = XT[0:128]
        big[:, N : 2 * N] = XT[128:256]
        big[:, XTC0 : XTC0 + ROWS] = XT[0:128][:, rows]
        big[:, XTC0 + ROWS : XTC0 + 2 * ROWS] = XT[128:256][:, rows]
        big[:, XTCP0 : XTCP0 + ROWS] = XT[0:128][:, rows] + 1.0
        big[:, XTCP0 + ROWS : XTCP0 + 2 * ROWS] = XT[128:256][:, rows] + 1.0
        big[0:ROWS, ONES0] = 1.0
        same = lab[rows][:, None] == lab[None, :]
        big[0:ROWS, AM0 : AM0 + N] = same.astype(np.float32)
        in_maps.append(dict(big=big, wselb=wselb))
    return in_maps


def host_finish(X, lab, SA, SB):
    """Combine device partials (SA = sum_{same ordered} (G+P), SB =
    sum_{all ordered} (G+P)) into the three losses, in f64."""
    Xd = X.astype(np.float64)
    s = (Xd * Xd).sum(axis=1)
    Ssum = s.sum()
    labs, counts = np.unique(lab, return_counts=True)
    Sl = np.array([s[lab == l].sum() for l in labs])
    n1 = int((counts.astype(np.int64) ** 2).sum())
    n2 = N * N - n1

    inner_sum = ((counts * Sl).sum() - SA) / D
    total_sum = (N * Ssum - SB) / D
    outer_sum = total_sum - inner_sum

    loss_inner = inner_sum / n1 if n1 > 0 else inner_sum
    loss_outer = outer_sum / max(n2, 1) if n2 > 0 else outer_sum
    penalty = ((np.sqrt(s) - 10.0) ** 2).mean()
    return (
        np.float32(loss_inner),
        np.float32(loss_outer),
        np.float32(penalty),
    )


def kernel(distributions, labels):
    from concourse.bass_utils import run_bass_kernel_spmd

    X = np.asarray(distributions, dtype=np.float32)
    lab = np.asarray(labels).astype(np.int64)
    assert X.shape == (N, D), X.shape

    if "nc" not in _NC_CACHE:
        _NC_CACHE["nc"] = build_nc()
    nc = _NC_CACHE["nc"]

    in_maps = prepare_in_maps(X, lab)
    results = run_bass_kernel_spmd(nc, in_maps, list(range(NCORES))).results
    SA = float(sum(np.float64(r["out"][0, 0]) for r in results))
    SB = float(sum(np.float64(r["out"][0, 1]) for r in results))
    return host_finish(X, lab, SA, SB)


# revision 35
# speedup vs baseline: 4.6638x; 4.6638x over previous
"""Trainium2 Bass kernel for nn_Aggregation_Separation_Loss.

Math: pairwise SmoothL1 (beta=1, mean over D) for all (i,j):
    huber(z) = 0.5*z^2 - 0.5*relu(|z|-1)^2
    sl1[i,j]*D = 0.5*s_i + 0.5*s_j - G_ij - 0.5*V_ij
with s_i = ||x_i||^2, G = X X^T (TensorE matmul), and
V_ij = sum_d relu(|x_id-x_jd|-1)^2 the only O(N^2 D) elementwise part.

Key symmetry: relu(|z|-1)^2 = relu(z-1)^2 + relu(-z-1)^2, and the second
term of pair (i,j) is the first term of (j,i).  So with the one-sided
P_ij = sum_d relu(x_i - x_j - 1)^2 over ALL ordered pairs:
    sum_{S symmetric} 0.5*V = sum_S P   (diag: P_ii = 0).
Per (row i, d-tile of 128) unit:
    DVE :  u = (Xt - x_i) max 1.0      (one fused tensor_scalar, 2x mode)
    ACT :  v = Square(u - 1) = relu(z-1)^2    (bias = -1)
    PE  :  selector-matmul contracts v over d into PSUM row i -> P[i, :]
G goes to a second PSUM tile; on-device masked reductions give per core
    SA = sum_{same-label ordered} (G + P),  SB = sum_{all ordered} (G + P)
and the host finishes with closed forms in f64:
    inner_sum = (sum_c N_c*S_c - SA_tot) / D
    total_sum = (N*sum(s)  - SB_tot) / D.

Sharding: plain row-parallel, core c owns rows [96c, 96c+96); every unit
is full width so the SPMD program is uniform across cores.
"""

import numpy as np

import concourse.bass as bass
import concourse.mybir as mybir
import concourse.tile as tile
from concourse.bacc import Bacc

N = 768
D = 256
NCORES = 8
ROWS = 96
DT = 2  # d-tiles of 128 partitions
F32 = mybir.dt.float32
BF16 = mybir.dt.bfloat16
# f32 input column layout ("big"): per-row scalars + masks (finals)
ONES0 = 0
AM0 = 1
BW = AM0 + N + 1                  # 770 (even)
# bf16 input column layout ("bigb"): selectors + heavy/G operands
XTB0 = 1024                       # xt (bf16) d-tiles
XTCB0 = XTB0 + 2 * N              # 2560: xtc (bf16, G lhsT)
BWB = XTCB0 + 2 * ROWS            # 2752

_NC_CACHE = {}


def _chunks():
    """Split [0, N) at the PSUM bank boundary (512 f32)."""
    return [(0, 512), (512, N)]


def build_nc(heavy_reps=1):
    # heavy_reps > 1 repeats the heavy loop for wall-clock timing via
    # repetition amplification (output is then wrong; timing only).
    #
    # Bacc (not plain Bass): its finalize() runs move_matmul_waits_to_
    # ldweights + generate_event_semaphores, which legalize multi-wait
    # instructions down to the 1-sync-wait-per-instruction HW limit.
    #
    # Inputs ride in three tensors (few DMAs -> few cross-engine waits):
    #   bigb (bf16): wsel selectors | xt d-tiles | xtc (G lhsT)
    #   xtcpf (f32): per-row scalars x_i + 1 for the heavy loop
    #   big  (f32):  ones column | same-label mask (finals only)
    nc = Bacc()
    big_d = nc.dram_tensor("big", [96, BW], F32, kind="ExternalInput")
    bigb_d = nc.dram_tensor("bigb", [128, BWB], BF16, kind="ExternalInput")
    xtcpf_d = nc.dram_tensor("xtcpf", [128, 2 * ROWS], F32, kind="ExternalInput")
    out_d = nc.dram_tensor("out", [1, 2], F32, kind="ExternalOutput")

    with tile.TileContext(nc) as tc:
        with (
            tc.tile_pool(name="pers", bufs=1) as pers,
            tc.tile_pool(name="tt", bufs=6) as tpool,
            tc.tile_pool(name="vv", bufs=6) as vpool,
            tc.tile_pool(name="fin", bufs=1) as fin,
            tc.tile_pool(name="psum", bufs=1, space=bass.MemorySpace.PSUM) as psum,
        ):
            big = pers.tile([96, BW], F32, tag="big")
            bigb = pers.tile([128, BWB], BF16, tag="bigb")
            xtcpf = pers.tile([128, 2 * ROWS], F32, tag="xtcpf")
            # heavy loop needs only xtcpf + bigb; the big f32 tensor (G
            # operands + masks) can land later while the loop runs.
            nc.gpsimd.dma_start(xtcpf[:], xtcpf_d[:])
            nc.gpsimd.dma_start(bigb[:], bigb_d[:])
            nc.gpsimd.dma_start(big[:], big_d[:])

            # wsel[:, 32r:32r+32] is a [128,32] selector: column r = 1.0,
            # rest 0.  As matmul lhsT it adds sum_d(rhs) into psum row r of
            # a 32-row block and 0 into the other 31 rows (PE psum outputs
            # must start at partition 0/32/64, so m=1 writes at arbitrary
            # partitions are not allowed).
            ones96 = big[0:ROWS, ONES0 : ONES0 + 1]
            am = big[0:ROWS, AM0 : AM0 + N]
            xtcp = [xtcpf[:, 0:ROWS], xtcpf[:, ROWS : 2 * ROWS]]
            wselb = bigb[:, 0:1024]
            xtb = [bigb[:, XTB0 : XTB0 + N], bigb[:, XTB0 + N : XTB0 + 2 * N]]
            xtcb = [
                bigb[:, XTCB0 : XTCB0 + ROWS],
                bigb[:, XTCB0 + ROWS : XTCB0 + 2 * ROWS],
            ]

            pg = psum.tile([ROWS, N], F32, tag="pg")
            pv = psum.tile([ROWS, N], F32, tag="pv")
            sab = psum.tile([1, 2], F32, tag="sab")

            # G = X_rows @ X^T accumulated over the two d-tiles (bf16).
            for c0, c1 in _chunks():
                for t in range(DT):
                    nc.tensor.matmul(
                        pg[:, c0:c1],
                        xtcb[t][:],
                        xtb[t][:, c0:c1],
                        start=(t == 0),
                        stop=(t == DT - 1),
                    )

            # Heavy loop: P rows into pv.  Per unit (row, d-tile):
            #   u = relu(x_j - x_i - 1) = (xtb - (x_i+1)) max 0  [DVE, bf16
            #       single-src 4x mode]
            #   v = u*u (bf16)      [ACT Square, or DVE TT 2x for ~2/5]
            #   selector-matmul v over d into psum row           [PE]
            # GPSIMD is deliberately NOT used for elementwise work: its
            # CoreSim cost is optimistic but real-HW throughput is several
            # times worse (measured 465us vs 217us baseline).
            first_touch = set()
            for rep in range(heavy_reps):
                for row in range(ROWS):
                    b, r = divmod(row, 32)
                    # both d-tiles' u land side by side in one [128, 2N]
                    # tile so a single square instruction covers the row
                    # (halves ACT/DVE per-instruction overhead)
                    dve_sq = (row * 38) // ROWS != ((row + 1) * 38) // ROWS
                    tt = tpool.tile(
                        [128, 2 * N], BF16, tag="tt", name=f"tt_{rep}_{row}"
                    )
                    for t in range(DT):
                        nc.vector.tensor_scalar(
                            tt[:, t * N : (t + 1) * N],
                            xtb[t][:],
                            xtcp[t][:, row : row + 1],
                            0.0,
                            op0=mybir.AluOpType.subtract,
                            op1=mybir.AluOpType.max,
                        )
                    vv = vpool.tile(
                        [128, 2 * N], BF16, tag="vv", name=f"vv_{rep}_{row}"
                    )
                    if dve_sq:
                        nc.vector.tensor_tensor(
                            vv[:], tt[:], tt[:], op=mybir.AluOpType.mult
                        )
                    else:
                        nc.scalar.activation(
                            vv[:],
                            tt[:],
                            mybir.ActivationFunctionType.Square,
                            bias=0.0,
                            scale=1.0,
                        )
                    for t in range(DT):
                        for c0, c1 in _chunks():
                            # start=True resets the full 32-row psum block,
                            # so only the first touch of each (block, chunk)
                            # region may use it.
                            key = (b, c0)
                            nc.tensor.matmul(
                                pv[32 * b : 32 * b + 32, c0:c1],
                                wselb[:, 32 * r : 32 * r + 32],
                                vv[:, t * N + c0 : t * N + c1],
                                start=key not in first_touch,
                                stop=(rep == heavy_reps - 1 and t == DT - 1),
                                skip_group_check=True,
                            )
                            first_touch.add(key)

            # R = G + P ; SA = sum(R*am) ; SB = sum(R).  Plain TT +
            # reduce ops only (HW-proven).  TT reads at most one PSUM
            # input, so pv is staged through SBUF.
            red = fin.tile([ROWS, 2], F32, tag="red")
            r0 = fin.tile([ROWS, N], F32, tag="r0")
            nc.vector.tensor_scalar(r0[:], pv[:], 1.0, None, op0=mybir.AluOpType.mult)
            r1 = fin.tile([ROWS, N], F32, tag="r1")
            nc.vector.tensor_tensor(r1[:], r0[:], pg[:], op=mybir.AluOpType.add)
            prod = fin.tile([ROWS, N], F32, tag="prod")
            nc.vector.tensor_tensor(prod[:], r1[:], am[:], op=mybir.AluOpType.mult)
            nc.vector.tensor_reduce(
                red[:, 0:1], prod[:], axis=mybir.AxisListType.X,
                op=mybir.AluOpType.add,
            )
            nc.vector.tensor_reduce(
                red[:, 1:2], r1[:], axis=mybir.AxisListType.X,
                op=mybir.AluOpType.add,
            )
            nc.tensor.matmul(sab[:], ones96[:], red[:], start=True, stop=True)

            outsb = fin.tile([1, 2], F32, tag="outsb")
            nc.scalar.copy(outsb[:], sab[:])
            nc.gpsimd.dma_start(out_d[:], outsb[:])

    nc.finalize()
    return nc


def core_rows(c):
    return np.arange(ROWS * c, ROWS * (c + 1))


def prepare_in_maps(X, lab):
    """X: [N, D] f32, lab: [N] int -> list of per-core input dicts."""
    import ml_dtypes

    XT = np.ascontiguousarray(X.T)  # [D, N]
    in_maps = []
    for c in range(NCORES):
        rows = core_rows(c)
        big = np.zeros((96, BW), np.float32)
        big[0:ROWS, ONES0] = 1.0
        same = lab[rows][:, None] == lab[None, :]
        big[0:ROWS, AM0 : AM0 + N] = same.astype(np.float32)
        xtcpf = np.empty((128, 2 * ROWS), np.float32)
        xtcpf[:, 0:ROWS] = XT[0:128][:, rows] + 1.0
        xtcpf[:, ROWS : 2 * ROWS] = XT[128:256][:, rows] + 1.0

        bigb = np.zeros((128, BWB), dtype=ml_dtypes.bfloat16)
        for r in range(32):
            bigb[:, 32 * r + r] = 1.0
        bigb[:, XTB0 : XTB0 + N] = XT[0:128]
        bigb[:, XTB0 + N : XTB0 + 2 * N] = XT[128:256]
        bigb[:, XTCB0 : XTCB0 + ROWS] = XT[0:128][:, rows]
        bigb[:, XTCB0 + ROWS : XTCB0 + 2 * ROWS] = XT[128:256][:, rows]
        in_maps.append(dict(big=big, bigb=bigb, xtcpf=xtcpf))
    return in_maps


def host_finish(X, lab, SA, SB):
    """Combine device partials (SA = sum_{same ordered} (G+P), SB =
    sum_{all ordered} (G+P)) into the three losses, in f64."""
    Xd = X.astype(np.float64)
    s = (Xd * Xd).sum(axis=1)
    Ssum = s.sum()
    labs, counts = np.unique(lab, return_counts=True)
    Sl = np.array([s[lab == l].sum() for l in labs])
    n1 = int((counts.astype(np.int64) ** 2).sum())
    n2 = N * N - n1

    inner_sum = ((counts * Sl).sum() - SA) / D
    total_sum = (N * Ssum - SB) / D
    outer_sum = total_sum - inner_sum

    loss_inner = inner_sum / n1 if n1 > 0 else inner_sum
    loss_outer = outer_sum / max(n2, 1) if n2 > 0 else outer_sum
    penalty = ((np.sqrt(s) - 10.0) ** 2).mean()
    return (
        np.float32(loss_inner),
        np.float32(loss_outer),
        np.float32(penalty),
    )


def kernel(distributions, labels):
    from concourse.bass_utils import run_bass_kernel_spmd

    X = np.asarray(distributions, dtype=np.float32)
    lab = np.asarray(labels).astype(np.int64)
    assert X.shape == (N, D), X.shape

    if "nc" not in _NC_CACHE:
        _NC_CACHE["nc"] = build_nc()
    nc = _NC_CACHE["nc"]

    in_maps = prepare_in_maps(X, lab)
    results = run_bass_kernel_spmd(nc, in_maps, list(range(NCORES))).results
    SA = float(sum(np.float64(r["out"][0, 0]) for r in results))
    SB = float(sum(np.float64(r["out"][0, 1]) for r in results))
    return host_finish(X, lab, SA, SB)


# revision 40
# speedup vs baseline: 4.7075x; 1.0094x over previous
"""Trainium2 Bass kernel for nn_Aggregation_Separation_Loss.

Math: pairwise SmoothL1 (beta=1, mean over D) for all (i,j):
    huber(z) = 0.5*z^2 - 0.5*relu(|z|-1)^2
    sl1[i,j]*D = 0.5*s_i + 0.5*s_j - G_ij - 0.5*V_ij
with s_i = ||x_i||^2, G = X X^T (TensorE matmul), and
V_ij = sum_d relu(|x_id-x_jd|-1)^2 the only O(N^2 D) elementwise part.

Key symmetry: relu(|z|-1)^2 = relu(z-1)^2 + relu(-z-1)^2, and the second
term of pair (i,j) is the first term of (j,i).  So with the one-sided
P_ij = sum_d relu(x_i - x_j - 1)^2 over ALL ordered pairs:
    sum_{S symmetric} 0.5*V = sum_S P   (diag: P_ii = 0).
Per (row i, d-tile of 128) unit:
    DVE :  u = (Xt - x_i) max 1.0      (one fused tensor_scalar, 2x mode)
    ACT :  v = Square(u - 1) = relu(z-1)^2    (bias = -1)
    PE  :  selector-matmul contracts v over d into PSUM row i -> P[i, :]
G goes to a second PSUM tile; on-device masked reductions give per core
    SA = sum_{same-label ordered} (G + P),  SB = sum_{all ordered} (G + P)
and the host finishes with closed forms in f64:
    inner_sum = (sum_c N_c*S_c - SA_tot) / D
    total_sum = (N*sum(s)  - SB_tot) / D.

Sharding: plain row-parallel, core c owns rows [96c, 96c+96); every unit
is full width so the SPMD program is uniform across cores.
"""

import numpy as np

import concourse.bass as bass
import concourse.mybir as mybir
import concourse.tile as tile
from concourse.bacc import Bacc

N = 768
D = 256
NCORES = 8
ROWS = 96
DT = 2  # d-tiles of 128 partitions
F32 = mybir.dt.float32
BF16 = mybir.dt.bfloat16
# f32 input column layout ("big"): per-row scalars + masks (finals)
ONES0 = 0
AM0 = 1
BW = AM0 + N + 1                  # 770 (even)
# bf16 input column layout ("bigb"): selectors + heavy/G operands
XTB0 = 1024                       # xt (bf16) d-tiles
XTCB0 = XTB0 + 2 * N              # 2560: xtc (bf16, G lhsT)
BWB = XTCB0 + 2 * ROWS            # 2752

_NC_CACHE = {}


def _chunks():
    """Split [0, N) at the PSUM bank boundary (512 f32)."""
    return [(0, 512), (512, N)]


def build_nc(heavy_reps=1):
    # heavy_reps > 1 repeats the heavy loop for wall-clock timing via
    # repetition amplification (output is then wrong; timing only).
    #
    # Bacc (not plain Bass): its finalize() runs move_matmul_waits_to_
    # ldweights + generate_event_semaphores, which legalize multi-wait
    # instructions down to the 1-sync-wait-per-instruction HW limit.
    #
    # Inputs ride in three tensors (few DMAs -> few cross-engine waits):
    #   bigb (bf16): wsel selectors | xt d-tiles | xtc (G lhsT)
    #   xtcpf (f32): per-row scalars x_i + 1 for the heavy loop
    #   big  (f32):  ones column | same-label mask (finals only)
    nc = Bacc()
    big_d = nc.dram_tensor("big", [96, BW], F32, kind="ExternalInput")
    bigb_d = nc.dram_tensor("bigb", [128, BWB], BF16, kind="ExternalInput")
    xtcpf_d = nc.dram_tensor("xtcpf", [128, 2 * ROWS], F32, kind="ExternalInput")
    out_d = nc.dram_tensor("out", [1, 2], F32, kind="ExternalOutput")

    with tile.TileContext(nc) as tc:
        with (
            tc.tile_pool(name="pers", bufs=1) as pers,
            tc.tile_pool(name="tt", bufs=6) as tpool,
            tc.tile_pool(name="vv", bufs=6) as vpool,
            tc.tile_pool(name="fin", bufs=1) as fin,
            tc.tile_pool(name="psum", bufs=1, space=bass.MemorySpace.PSUM) as psum,
        ):
            big = pers.tile([96, BW], F32, tag="big")
            bigb = pers.tile([128, BWB], BF16, tag="bigb")
            xtcpf = pers.tile([128, 2 * ROWS], F32, tag="xtcpf")
            # heavy loop needs only xtcpf + bigb; the big f32 tensor (G
            # operands + masks) can land later while the loop runs.
            nc.gpsimd.dma_start(xtcpf[:], xtcpf_d[:])
            nc.gpsimd.dma_start(bigb[:], bigb_d[:])
            nc.gpsimd.dma_start(big[:], big_d[:])

            # wsel[:, 32r:32r+32] is a [128,32] selector: column r = 1.0,
            # rest 0.  As matmul lhsT it adds sum_d(rhs) into psum row r of
            # a 32-row block and 0 into the other 31 rows (PE psum outputs
            # must start at partition 0/32/64, so m=1 writes at arbitrary
            # partitions are not allowed).
            ones96 = big[0:ROWS, ONES0 : ONES0 + 1]
            am = big[0:ROWS, AM0 : AM0 + N]
            xtcp = [xtcpf[:, 0:ROWS], xtcpf[:, ROWS : 2 * ROWS]]
            wselb = bigb[:, 0:1024]
            xtb = [bigb[:, XTB0 : XTB0 + N], bigb[:, XTB0 + N : XTB0 + 2 * N]]
            xtcb = [
                bigb[:, XTCB0 : XTCB0 + ROWS],
                bigb[:, XTCB0 + ROWS : XTCB0 + 2 * ROWS],
            ]

            pg = psum.tile([ROWS, N], F32, tag="pg")
            pv = psum.tile([ROWS, N], F32, tag="pv")
            sab = psum.tile([1, 2], F32, tag="sab")

            # G = X_rows @ X^T accumulated over the two d-tiles (bf16).
            for c0, c1 in _chunks():
                for t in range(DT):
                    nc.tensor.matmul(
                        pg[:, c0:c1],
                        xtcb[t][:],
                        xtb[t][:, c0:c1],
                        start=(t == 0),
                        stop=(t == DT - 1),
                    )

            # Stage G into SBUF now: the copy runs hidden under the
            # heavy loop, and the finals' TT add can then pair PSUM pv
            # with SBUF pgs directly (TT reads at most one PSUM operand).
            pgs = fin.tile([ROWS, N], F32, tag="pgs")
            nc.vector.tensor_scalar(
                pgs[:], pg[:], 1.0, None, op0=mybir.AluOpType.mult
            )

            # Heavy loop: P rows into pv.  Per unit (row, d-tile):
            #   u = relu(x_j - x_i - 1) = (xtb - (x_i+1)) max 0  [DVE, bf16
            #       single-src 4x mode]
            #   v = u*u (bf16)      [ACT Square, or DVE TT 2x for ~2/5]
            #   selector-matmul v over d into psum row           [PE]
            # GPSIMD is deliberately NOT used for elementwise work: its
            # CoreSim cost is optimistic but real-HW throughput is several
            # times worse (measured 465us vs 217us baseline).
            first_touch = set()
            for rep in range(heavy_reps):
                for row in range(ROWS):
                    b, r = divmod(row, 32)
                    # both d-tiles' u land side by side in one [128, 2N]
                    # tile so a single square instruction covers the row
                    # (halves ACT/DVE per-instruction overhead)
                    dve_sq = (row * 38) // ROWS != ((row + 1) * 38) // ROWS
                    tt = tpool.tile(
                        [128, 2 * N], BF16, tag="tt", name=f"tt_{rep}_{row}"
                    )
                    for t in range(DT):
                        nc.vector.tensor_scalar(
                            tt[:, t * N : (t + 1) * N],
                            xtb[t][:],
                            xtcp[t][:, row : row + 1],
                            0.0,
                            op0=mybir.AluOpType.subtract,
                            op1=mybir.AluOpType.max,
                        )
                    vv = vpool.tile(
                        [128, 2 * N], BF16, tag="vv", name=f"vv_{rep}_{row}"
                    )
                    if dve_sq:
                        nc.vector.tensor_tensor(
                            vv[:], tt[:], tt[:], op=mybir.AluOpType.mult
                        )
                    else:
                        nc.scalar.activation(
                            vv[:],
                            tt[:],
                            mybir.ActivationFunctionType.Square,
                            bias=0.0,
                            scale=1.0,
                        )
                    for t in range(DT):
                        for c0, c1 in _chunks():
                            # start=True resets the full 32-row psum block,
                            # so only the first touch of each (block, chunk)
                            # region may use it.
                            key = (b, c0)
                            nc.tensor.matmul(
                                pv[32 * b : 32 * b + 32, c0:c1],
                                wselb[:, 32 * r : 32 * r + 32],
                                vv[:, t * N + c0 : t * N + c1],
                                start=key not in first_touch,
                                stop=(rep == heavy_reps - 1 and t == DT - 1),
                                skip_group_check=True,
                            )
                            first_touch.add(key)

            # R = G + P ; SA = sum(R*am) ; SB = sum(R).  Plain TT +
            # reduce ops only (HW-proven); pg was staged to SBUF above.
            red = fin.tile([ROWS, 2], F32, tag="red")
            r1 = fin.tile([ROWS, N], F32, tag="r1")
            nc.vector.tensor_tensor(r1[:], pv[:], pgs[:], op=mybir.AluOpType.add)
            prod = fin.tile([ROWS, N], F32, tag="prod")
            nc.vector.tensor_tensor(prod[:], r1[:], am[:], op=mybir.AluOpType.mult)
            nc.vector.tensor_reduce(
                red[:, 0:1], prod[:], axis=mybir.AxisListType.X,
                op=mybir.AluOpType.add,
            )
            nc.vector.tensor_reduce(
                red[:, 1:2], r1[:], axis=mybir.AxisListType.X,
                op=mybir.AluOpType.add,
            )
            nc.tensor.matmul(sab[:], ones96[:], red[:], start=True, stop=True)

            outsb = fin.tile([1, 2], F32, tag="outsb")
            nc.scalar.copy(outsb[:], sab[:])
            nc.gpsimd.dma_start(out_d[:], outsb[:])

    nc.finalize()
    return nc


def core_rows(c):
    return np.arange(ROWS * c, ROWS * (c + 1))


def prepare_in_maps(X, lab):
    """X: [N, D] f32, lab: [N] int -> list of per-core input dicts."""
    import ml_dtypes

    XT = np.ascontiguousarray(X.T)  # [D, N]
    in_maps = []
    for c in range(NCORES):
        rows = core_rows(c)
        big = np.zeros((96, BW), np.float32)
        big[0:ROWS, ONES0] = 1.0
        same = lab[rows][:, None] == lab[None, :]
        big[0:ROWS, AM0 : AM0 + N] = same.astype(np.float32)
        xtcpf = np.empty((128, 2 * ROWS), np.float32)
        xtcpf[:, 0:ROWS] = XT[0:128][:, rows] + 1.0
        xtcpf[:, ROWS : 2 * ROWS] = XT[128:256][:, rows] + 1.0

        bigb = np.zeros((128, BWB), dtype=ml_dtypes.bfloat16)
        for r in range(32):
            bigb[:, 32 * r + r] = 1.0
        bigb[:, XTB0 : XTB0 + N] = XT[0:128]
        bigb[:, XTB0 + N : XTB0 + 2 * N] = XT[128:256]
        bigb[:, XTCB0 : XTCB0 + ROWS] = XT[0:128][:, rows]
        bigb[:, XTCB0 + ROWS : XTCB0 + 2 * ROWS] = XT[128:256][:, rows]
        in_maps.append(dict(big=big, bigb=bigb, xtcpf=xtcpf))
    return in_maps


def host_finish(X, lab, SA, SB):
    """Combine device partials (SA = sum_{same ordered} (G+P), SB =
    sum_{all ordered} (G+P)) into the three losses, in f64."""
    Xd = X.astype(np.float64)
    s = (Xd * Xd).sum(axis=1)
    Ssum = s.sum()
    labs, counts = np.unique(lab, return_counts=True)
    Sl = np.array([s[lab == l].sum() for l in labs])
    n1 = int((counts.astype(np.int64) ** 2).sum())
    n2 = N * N - n1

    inner_sum = ((counts * Sl).sum() - SA) / D
    total_sum = (N * Ssum - SB) / D
    outer_sum = total_sum - inner_sum

    loss_inner = inner_sum / n1 if n1 > 0 else inner_sum
    loss_outer = outer_sum / max(n2, 1) if n2 > 0 else outer_sum
    penalty = ((np.sqrt(s) - 10.0) ** 2).mean()
    return (
        np.float32(loss_inner),
        np.float32(loss_outer),
        np.float32(penalty),
    )


def kernel(distributions, labels):
    from concourse.bass_utils import run_bass_kernel_spmd

    X = np.asarray(distributions, dtype=np.float32)
    lab = np.asarray(labels).astype(np.int64)
    assert X.shape == (N, D), X.shape

    if "nc" not in _NC_CACHE:
        _NC_CACHE["nc"] = build_nc()
    nc = _NC_CACHE["nc"]

    in_maps = prepare_in_maps(X, lab)
    results = run_bass_kernel_spmd(nc, in_maps, list(range(NCORES))).results
    SA = float(sum(np.float64(r["out"][0, 0]) for r in results))
    SB = float(sum(np.float64(r["out"][0, 1]) for r in results))
    return host_finish(X, lab, SA, SB)
